# revision 5
# baseline (speedup 1.0000x reference)
"""Trainium2 Bass kernel for nn_MeanPooling (segment_reduce).

Computes out[b,e,h] = (sum_l entity_mapping[b,e,l] * doc_state[b,l,h]) / entity_lens[b,e]
for B=16, E=128, L=2048, H=1024.

Sharding: data-parallel over batch B across 8 NeuronCores (2 batches per core).
Per core, each batch is a (E=128, L=2048) @ (L=2048, H=1024) matmul:
  - entity_mapping[b] is DMA'd naturally (E on partitions) and transposed
    128x128-tile-wise on the TensorEngine (contraction dim L must be on
    partitions for both matmul operands).
  - doc_state[b] is DMA'd naturally (L on partitions).
  - 16 k-tile matmuls accumulate into 2 PSUM banks (N=512 each).
  - The divide by entity_lens is fused into the PSUM->SBUF eviction as an
    ACT activation(Copy, scale=1/lens) with a per-partition scale.
"""

import os

import numpy as np

B, E, L, H = 16, 128, 2048, 1024
N_CORES = 8
B_PER_CORE = B // N_CORES
P = 128
KT = L // P  # 16 k-tiles
DOC_CHUNK = 2  # k-tiles per doc dma (1 MiB)

# matmul dtype flavor: "f32" (bit-accurate, 4 cyc/row) or "f32r" (full rate at
# N>=256; numerics verified against the f32 reference in test.py)
MM_FLAVOR = os.environ.get("BASS_MM_FLAVOR", "f32r")

_CACHE = {}


def _build_bass():
    import concourse.mybir as mybir
    from concourse import bacc
    from concourse.bass import ts
    from concourse.masks import make_identity
    from concourse.tile import TileContext

    f32 = mybir.dt.float32
    mm_dt = {"f32": mybir.dt.float32, "f32r": mybir.dt.float32r}[MM_FLAVOR]

    nc = bacc.Bacc(None, target_bir_lowering=False)
    doc = nc.dram_tensor("doc_state", [B_PER_CORE, L, H], f32, kind="ExternalInput")
    mp = nc.dram_tensor("entity_mapping", [B_PER_CORE, E, L], f32, kind="ExternalInput")
    lens = nc.dram_tensor("entity_lens", [B_PER_CORE, E], f32, kind="ExternalInput")
    out = nc.dram_tensor("out", [B_PER_CORE, E, H], f32, kind="ExternalOutput")

    lens_cols = lens.rearrange("b e -> e b")  # (E, B_PER_CORE) in DRAM

    with TileContext(nc) as tc:
        with (
            tc.tile_pool(name="const", bufs=1) as const_pool,
            tc.tile_pool(name="mapp", bufs=2) as map_pool,
            tc.tile_pool(name="mapt", bufs=2) as mapt_pool,
            tc.tile_pool(name="doc", bufs=8) as doc_pool,
            tc.tile_pool(name="outp", bufs=2) as out_pool,
            tc.tile_pool(name="lens", bufs=4) as lens_pool,
            tc.tile_pool(name="psum", bufs=2, space="PSUM") as psum_pool,
            tc.tile_pool(name="psumt", bufs=4, space="PSUM") as psumt_pool,
        ):
            identity = const_pool.tile([P, P], f32)
            make_identity(nc, identity)

            for b in range(B_PER_CORE):
                # --- per-entity reciprocal lengths (128 x 1) ---
                lens_sb = lens_pool.tile([E, 1], f32, tag="lens_sb")
                nc.sync.dma_start(out=lens_sb, in_=lens_cols[:, b : b + 1])
                recip_sb = lens_pool.tile([E, 1], f32, tag="recip_sb")
                nc.vector.reciprocal(recip_sb, lens_sb)

                # --- mapping: load (E,L) naturally, PE-transpose to (L,E) tiles ---
                map_sb = map_pool.tile([E, L], f32)
                nc.sync.dma_start(out=map_sb, in_=mp[b])
                mapt_sb = mapt_pool.tile([P, KT, E], f32)
                for k in range(KT):
                    ps_t = psumt_pool.tile([P, E], f32)
                    nc.tensor.transpose(ps_t, map_sb[:, ts(k, P)], identity)
                    nc.vector.tensor_copy(mapt_sb[:, k, :], ps_t)

                # --- doc: (L,H) -> (P, KT, H), chunked DMAs ---
                doc_r = doc[b].rearrange("(ko p) h -> p ko h", p=P)
                doc_tiles = []
                for j in range(KT // DOC_CHUNK):
                    dtile = doc_pool.tile([P, DOC_CHUNK, H], f32)
                    nc.sync.dma_start(out=dtile, in_=doc_r[:, ts(j, DOC_CHUNK), :])
                    doc_tiles.append(dtile)

                # --- matmul: accumulate 16 k-tiles into 2 PSUM banks ---
                out_sb = out_pool.tile([E, H], f32)
                psums = [
                    psum_pool.tile([E, 512], f32, name=f"psum_{n}") for n in range(2)
                ]
                for k in range(KT):
                    rhs_tile = doc_tiles[k // DOC_CHUNK][:, k % DOC_CHUNK, :]
                    for n in range(2):
                        nc.tensor.matmul(
                            psums[n],
                            lhsT=mapt_sb[:, k, :].bitcast(mm_dt),
                            rhs=rhs_tile[:, ts(n, 512)].bitcast(mm_dt),
                            start=(k == 0),
                            stop=(k == KT - 1),
                        )
                for n in range(2):
                    # out = psum * (1/lens), fused PSUM->SBUF eviction on ACT
                    nc.scalar.activation(
                        out_sb[:, ts(n, 512)],
                        psums[n],
                        mybir.ActivationFunctionType.Copy,
                        scale=recip_sb,
                    )
                nc.sync.dma_start(out=out[b], in_=out_sb)

    nc.finalize()
    return nc


def _get_nc():
    if "nc" not in _CACHE:
        _CACHE["nc"] = _build_bass()
    return _CACHE["nc"]


def kernel(doc_state, entity_mapping, entity_lens, **run_kwargs):
    from concourse.bass_utils import run_bass_kernel_spmd

    nc = _get_nc()
    in_maps = []
    for i in range(N_CORES):
        sl = slice(i * B_PER_CORE, (i + 1) * B_PER_CORE)
        in_maps.append(
            {
                "doc_state": np.ascontiguousarray(doc_state[sl], dtype=np.float32),
                "entity_mapping": np.ascontiguousarray(
                    entity_mapping[sl], dtype=np.float32
                ),
                "entity_lens": np.ascontiguousarray(entity_lens[sl], dtype=np.float32),
            }
        )
    res = run_bass_kernel_spmd(nc, in_maps, core_ids=list(range(N_CORES)), **run_kwargs)
    out = np.concatenate([r["out"] for r in res.results], axis=0)
    if run_kwargs:
        _CACHE["last_result"] = res
    return out
